# revision 21
# baseline (speedup 1.0000x reference)
"""Trainium2 Bass kernel for nn_Attend (pooling/softmax-attention pooling).

Computation (per token row t, HID=256):
  z = x @ W.T + b ; t = tanh(z) ; s = c * t ; alpha = softmax_h(s)
  att = x * alpha ; out[n, :] = sum_w att[(n, w), :]

Data-parallel across 8 NeuronCores over the leading token axis.

Per-core pipeline (blocks of 512 tokens):
  1. SWDGE DMA loads x f32->bf16 (cast in flight), natural layout.
  2. PE transposes x -> xT (bf16 PSUM), ACT+DVE copy PSUM->SBUF.
  3. PE matmul z_T = Wt @ xT (T-layout: hidden on partitions).
  4. ACT tanh(z + b) with per-partition bias -> t (bf16).
  5. PE matmul t.T @ diag(c): transposes back to N-layout AND applies the
     context scale in one pass -> s_N (f32 PSUM).
  6. ACT exp(s_N) with accum_out -> e (bf16) + S (softmax denominator) free.
  7. DVE reciprocal r = 1/S ; att_u = x * e (bf16, unnormalized).
  8. Word-sum as PE matmul with mask'[p, g] = r[p] * onehot(group(p) == g):
     folds the 1/S normalization into the pooling matmul weights.
"""

import os
import sys
import numpy as np

try:
    import concourse.bass  # noqa: F401  (already on path?)
except ImportError:
    for _p in ("/opt/trn_rl_repo", "/root/.axon_site/_ro/trn_rl_repo"):
        if os.path.isdir(_p):
            sys.path.insert(0, _p)
            break

BATCH, HID, SENT, WORDS = 32, 256, 64, 64
N = BATCH * SENT                # 2048 sentences
NCORES = 8
NSH = N // NCORES               # 256 sentences per core
TOK = NSH * WORDS               # 16384 tokens per core
BLK = 512                       # tokens per block
NBLK = TOK // BLK               # 32 blocks
GRPB = 4                        # blocks per output group (32 sentences)

_CACHE = {}


def _build_module():
    from contextlib import ExitStack
    import concourse.bacc as bacc
    import concourse.mybir as mybir
    from concourse.tile import TileContext

    f32 = mybir.dt.float32
    bf16 = mybir.dt.bfloat16
    Alu = mybir.AluOpType
    Act = mybir.ActivationFunctionType

    nc = bacc.Bacc("TRN2", target_bir_lowering=False, debug=False,
                   num_devices=NCORES)

    x_d = nc.dram_tensor("x", [TOK, HID], f32, kind="ExternalInput")
    wt_d = nc.dram_tensor("wt", [128, 2, HID], bf16, kind="ExternalInput")
    b_d = nc.dram_tensor("bv", [128, 2], f32, kind="ExternalInput")
    dc_d = nc.dram_tensor("diagc", [128, 2, 128], bf16, kind="ExternalInput")
    id_d = nc.dram_tensor("ident", [128, 128], bf16, kind="ExternalInput")
    mk_d = nc.dram_tensor("mask01", [128, 16, 32], bf16, kind="ExternalInput")
    out_d = nc.dram_tensor("out", [NSH, HID], f32, kind="ExternalOutput")

    with TileContext(nc) as tc, ExitStack() as ctx:
        cpool = ctx.enter_context(tc.tile_pool(name="consts", bufs=1))
        xpool = ctx.enter_context(tc.tile_pool(name="xin", bufs=6))
        spool = ctx.enter_context(tc.tile_pool(name="work", bufs=4))
        opool = ctx.enter_context(tc.tile_pool(name="outs", bufs=2))
        pps = ctx.enter_context(tc.tile_pool(name="ps", bufs=1, space="PSUM"))
        ppx = ctx.enter_context(tc.tile_pool(name="psx", bufs=1, space="PSUM"))

        # ---- constants into SBUF ----
        wt_sb = cpool.tile([128, 2, HID], bf16, tag="wt")
        nc.sync.dma_start(wt_sb[:], wt_d.ap())
        b_sb = cpool.tile([128, 2], f32, tag="bv")
        nc.sync.dma_start(b_sb[:], b_d.ap())
        dc_sb = cpool.tile([128, 2, 128], bf16, tag="dc")
        nc.sync.dma_start(dc_sb[:], dc_d.ap())
        id_sb = cpool.tile([128, 128], bf16, tag="id")
        nc.sync.dma_start(id_sb[:], id_d.ap())
        mk_sb = cpool.tile([128, 16, 32], bf16, tag="mk")
        nc.sync.dma_start(mk_sb[:], mk_d.ap())

        repeat = int(os.environ.get("KERNEL_REPEAT", "1"))
        ws = None
        for _rep in range(repeat):
            _run_blocks(nc, tc, mybir, locals())

    nc.compile()
    return nc


def _run_blocks(nc, tc, mybir, env):
    f32 = mybir.dt.float32
    bf16 = mybir.dt.bfloat16
    Alu = mybir.AluOpType
    Act = mybir.ActivationFunctionType
    xpool, spool, opool, pps, ppx = (env["xpool"], env["spool"], env["opool"],
                                     env["pps"], env["ppx"])
    wt_sb, b_sb, dc_sb, id_sb, mk_sb = (env["wt_sb"], env["b_sb"],
                                        env["dc_sb"], env["id_sb"],
                                        env["mk_sb"])
    x_d, out_d = env["x_d"], env["out_d"]

    if True:
        ws = None
        for blk in range(NBLK):
            bb = blk % GRPB
            grp = blk // GRPB
            t0 = blk * BLK

            # 1. load + cast x block: token t0+128*j+p -> [p, j, h]
            xn = xpool.tile([128, 4, HID], bf16, tag="xn")
            src = x_d.ap()[t0:t0 + BLK, :].rearrange("(j p) h -> p j h", p=128)
            nc.gpsimd.dma_start(xn[:], src)

            # 2. transpose x -> xT (bf16 PSUM), then copy to SBUF
            xtp = ppx.tile([128, 2, BLK], bf16, tag="xtp")
            for j in range(4):
                for hc in range(2):
                    nc.tensor.transpose(
                        xtp[:, hc, 128 * j:128 * (j + 1)],
                        xn[:, j, 128 * hc:128 * (hc + 1)],
                        id_sb[:],
                    )
            xt = spool.tile([128, 2, BLK], bf16, tag="xt")
            nc.vector.tensor_copy(xt[:], xtp[:])

            # 3+4. z = Wt @ xT ; t = tanh(z + b)
            t_sb = spool.tile([128, 2, BLK], bf16, tag="t")
            for oc in range(2):
                zp = pps.tile([128, BLK], f32, tag=f"z{oc}", bufs=2)
                for hc in range(2):
                    nc.tensor.matmul(
                        zp[:],
                        wt_sb[:, hc, 128 * oc:128 * (oc + 1)],
                        xt[:, hc, :],
                        start=(hc == 0), stop=(hc == 1),
                    )
                nc.scalar.activation(
                    t_sb[:, oc, :], zp[:], Act.Tanh,
                    bias=b_sb[:, oc:oc + 1], scale=1.0,
                )

            # 5. s_N = t.T @ diag(c)  (transpose-back + context scale)
            sn = [pps.tile([128, 2, HID], f32, tag="snA", name="snA", bufs=1),
                  pps.tile([128, 2, HID], f32, tag="snB", name="snB", bufs=1)]
            for j in range(4):
                for oc in range(2):
                    nc.tensor.matmul(
                        sn[j // 2][:, j % 2, 128 * oc:128 * (oc + 1)],
                        t_sb[:, oc, 128 * j:128 * (j + 1)],
                        dc_sb[:, oc, :],
                        start=True, stop=True,
                    )

            # 6. e = exp(s_N); S = sum_h e via the ACT accumulate path
            S_sb = spool.tile([128, 4], f32, tag="S")
            e_sb = spool.tile([128, 4, HID], bf16, tag="e")
            for j in range(4):
                nc.scalar.activation(
                    e_sb[:, j, :], sn[j // 2][:, j % 2, :], Act.Exp,
                    accum_out=S_sb[:, j:j + 1],
                )

            # 7. r = 1/S ; att_u = x * e
            r_sb = spool.tile([128, 4], f32, tag="r")
            nc.vector.reciprocal(r_sb[:], S_sb[:])
            att = spool.tile([128, 4, HID], bf16, tag="att")
            nc.vector.tensor_tensor(att[:], xn[:], e_sb[:], Alu.mult)

            # 8. word-sum matmul with r folded into the mask weights
            if bb == 0:
                ws = pps.tile([32, HID], f32, tag="ws", bufs=1)
            for j in range(4):
                mr = spool.tile([128, 32], bf16, tag="mr")
                nc.vector.tensor_scalar(
                    mr[:], mk_sb[:, 4 * bb + j, :], r_sb[:, j:j + 1], None,
                    Alu.mult,
                )
                nc.tensor.matmul(
                    ws[:], mr[:], att[:, j, :],
                    start=(bb == 0 and j == 0), stop=(bb == 3 and j == 3),
                    skip_group_check=True,
                )

            if bb == GRPB - 1:
                osb = opool.tile([32, HID], f32, tag="osb")
                nc.vector.tensor_copy(osb[:], ws[:])
                nc.sync.dma_start(
                    out_d.ap()[32 * grp:32 * (grp + 1), :], osb[:])


def _host_consts():
    import ml_dtypes
    bf16 = ml_dtypes.bfloat16
    consts = _CACHE.get("consts")
    if consts is not None:
        return consts

    # mask01[p, 4*bb+j, g] = 1 iff g == 8*bb + 2*j + p//64
    mask01 = np.zeros((128, 16, 32), np.float32)
    for bb in range(4):
        for j in range(4):
            for p in range(128):
                mask01[p, 4 * bb + j, 8 * bb + 2 * j + p // 64] = 1.0
    _CACHE["consts"] = mask01.astype(bf16)
    return _CACHE["consts"]


def kernel(x, W, b, context, sentence_size):
    import ml_dtypes
    from concourse.bass_utils import run_bass_kernel_spmd

    bf16 = ml_dtypes.bfloat16
    key = "nc_r" + os.environ.get("KERNEL_REPEAT", "1")
    nc = _CACHE.get(key)
    if nc is None:
        nc = _build_module()
        _CACHE[key] = nc

    x = np.asarray(x, np.float32).reshape(N * WORDS, HID)
    W = np.asarray(W, np.float32)
    b = np.asarray(b, np.float32)
    c = np.asarray(context, np.float32)

    # wt[p, hc, o] = W[o, 128*hc + p]  (lhsT chunks, contraction dim h on p)
    wt = np.empty((128, 2, HID), np.float32)
    for hc in range(2):
        wt[:, hc, :] = W[:, 128 * hc:128 * (hc + 1)].T
    # b2[p, oc] = b[128*oc + p]
    b2 = b.reshape(2, 128).T.copy()
    # diagc[p, oc, n] = c[128*oc + n] if p == n else 0
    dc = np.zeros((128, 2, 128), np.float32)
    for oc in range(2):
        np.fill_diagonal(dc[:, oc, :], c[128 * oc:128 * (oc + 1)])
    ident = np.eye(128, dtype=np.float32)
    mask01 = _host_consts()

    shared = {
        "wt": wt.astype(bf16),
        "bv": np.ascontiguousarray(b2, np.float32),
        "diagc": dc.astype(bf16),
        "ident": ident.astype(bf16),
        "mask01": mask01,
    }
    in_maps = []
    for i in range(NCORES):
        m = dict(shared)
        m["x"] = np.ascontiguousarray(x[i * TOK:(i + 1) * TOK, :])
        in_maps.append(m)

    trace = bool(int(os.environ.get("KERNEL_TRACE", "0")))
    res = run_bass_kernel_spmd(nc, in_maps, core_ids=list(range(NCORES)),
                               trace=trace)
    _CACHE["last_results"] = res
    out = np.concatenate([res.results[i]["out"] for i in range(NCORES)], 0)
    return out.reshape(BATCH, SENT, HID).astype(np.float32)
